# revision 10
# baseline (speedup 1.0000x reference)
"""Cosine-similarity loss kernel for Trainium2 (8 NeuronCores, SPMD).

loss = -sum_n dot(s_n, im_n) / (||s_n|| * ||im_n||)   for s, im in R^{65536 x 512}

Strategy (memory-bound, ~358 GB/s HBM per core):
  - Shard the 65536 rows across 8 cores (8192 rows each, 32 MB/core streamed).
  - Per 128-row slice (64 slices/core), three fused one-pass reductions:
      dot = sum_d s*im  -> VectorE scalar_tensor_tensor (s*1)*im, accum_out
      ss  = sum_d s*s   -> ScalarE activation(Square, accum_out)
      ii  = sum_d im*im -> split DVE/ACT to balance effective per-op cost
                           (DVE ~776ns incl accum-read, ACT ~1182ns)
  - Tail (all in the ACT boot/default table -- Square+Sqrt share set 3, so
    no ACT_TABLE_LOAD switches):
      rsq = 1/sqrt(ss*ii) via ACT Sqrt + DVE reciprocal_approx_fast
      loss_p[128,1] = -sum_c dot*rsq   (DVE STT accum)
      scalar = ones^T @ loss_p via PE matmul -> PSUM[1,1]  (partition reduce)
      DMA 4 bytes out (single descriptor -> single completion receipt,
      vs [128,1] = 16 engines' straggling HBM-write receipts ~9us).
  - Host sums the 8 per-core scalars.
"""

import numpy as np
from contextlib import ExitStack

import concourse.bacc as bacc
import concourse.bass as bass
import concourse.mybir as mybir
import concourse.tile as tile
from concourse.bass_utils import run_bass_kernel_spmd

N, D = 65536, 512
N_CORES = 8
ROWS = N // N_CORES          # 8192 rows per core
P = 128                      # SBUF partitions
F32 = mybir.dt.float32


def _build(
    rows=ROWS,
    # slices per DMA tile (1 slice = 128 rows = 256KB/tensor).  Small first
    # tiles start compute early; small last tiles shrink the post-DMA tail.
    seg_schedule=(1, 1, 2) + (4,) * 13 + (2, 2, 1, 1, 1, 1),
    bufs=10,
    # which slices' ii goes to ACT: 5/16 spread evenly (DVE ~733ns/op vs ACT
    # ~940ns/op incl accum-read -> balance at DVE 108 ops / ACT 84 ops), and
    # never on the final slice so the last-arriving tile finishes fastest.
    ii_on_act=lambda c: c % 16 in (1, 4, 7, 10, 13),
    im_dma="scalar",         # engine issuing im loads: gpsimd | sync | scalar
    mapping="pj",            # row->partition: jp = row j*128+p; pj = p*seg+j
                             # (pj gives contiguous per-partition DMA segments)
):
    slices = rows // P
    assert sum(seg_schedule) == slices

    nc = bacc.Bacc(
        "TRN2", target_bir_lowering=False, debug=False, num_devices=N_CORES
    )
    s_d = nc.dram_tensor("s", [rows, D], F32, kind="ExternalInput").ap()
    im_d = nc.dram_tensor("im", [rows, D], F32, kind="ExternalInput").ap()
    out_d = nc.dram_tensor("out", [2, 1], F32, kind="ExternalOutput").ap()

    mult = mybir.AluOpType.mult

    with tile.TileContext(nc) as tc, ExitStack() as ctx:
        spool = ctx.enter_context(tc.tile_pool(name="spool", bufs=bufs))
        ipool = ctx.enter_context(tc.tile_pool(name="ipool", bufs=bufs))
        stats = ctx.enter_context(tc.tile_pool(name="stats", bufs=1))
        ppool = ctx.enter_context(
            tc.tile_pool(name="psum", bufs=1, space=bass.MemorySpace.PSUM)
        )

        dot_all = stats.tile([P, slices], F32)
        ss_all = stats.tile([P, slices], F32)
        ii_all = stats.tile([P, slices], F32)
        ones = stats.tile([P, 1], F32)
        nc.vector.memset(ones[:], 1.0)
        # First ACT instruction is a Sqrt (sqrt(1)=1, harmless): the
        # act-table pass then loads the sqrt_and_others set (which also
        # holds Square) once at entry, instead of a second 1.28us
        # ACT_TABLE_LOAD right before the tail's real Sqrt.
        nc.scalar.activation(ones[:], ones[:], mybir.ActivationFunctionType.Sqrt)
        dve_scr = stats.tile([P, D], F32)
        act_scr = stats.tile([P, D], F32)

        # tail math split in two chunks: chunk 0 (slices 0..split-1) is
        # emitted mid-stream right after its accums, fully hidden under the
        # DMA stream; only chunk 1 (the last slices) runs after the stream.
        split = 48
        prod = stats.tile([P, slices], F32)
        sqp = stats.tile([P, slices], F32)
        rsq = stats.tile([P, slices], F32)
        fin_scr = stats.tile([P, slices], F32)
        loss_p = stats.tile([P, 2], F32)

        def tail_chunk(idx, lo, hi):
            w = slice(lo, hi)
            nc.vector.tensor_tensor(
                out=prod[:, w], in0=ss_all[:, w], in1=ii_all[:, w], op=mult
            )
            nc.scalar.activation(
                sqp[:, w], prod[:, w], mybir.ActivationFunctionType.Sqrt
            )
            nc.vector.reciprocal_approx_fast(rsq[:, w], sqp[:, w])
            nc.vector.scalar_tensor_tensor(
                out=fin_scr[:, w], in0=dot_all[:, w], scalar=-1.0, in1=rsq[:, w],
                op0=mult, op1=mult,
                accum_out=loss_p[:, idx : idx + 1],
            )

        c = 0
        r0 = 0
        pat = "(j p) d -> p j d" if mapping == "jp" else "(p j) d -> p j d"
        for seg in seg_schedule:
            nrows = seg * P
            s_seg = s_d[r0 : r0 + nrows, :].rearrange(pat, p=P, j=seg)
            im_seg = im_d[r0 : r0 + nrows, :].rearrange(pat, p=P, j=seg)
            r0 += nrows
            st = spool.tile([P, seg, D], F32, name="st", tag="st")
            nc.sync.dma_start(st[:], s_seg)
            it = ipool.tile([P, seg, D], F32, name="it", tag="it")
            getattr(nc, im_dma).dma_start(it[:], im_seg)
            for j in range(seg):
                nc.vector.scalar_tensor_tensor(
                    out=dve_scr[:], in0=st[:, j, :], scalar=1.0, in1=it[:, j, :],
                    op0=mult, op1=mult,
                    accum_out=dot_all[:, c : c + 1],
                )
                nc.scalar.activation(
                    out=act_scr[:], in_=st[:, j, :],
                    func=mybir.ActivationFunctionType.Square,
                    accum_out=ss_all[:, c : c + 1],
                )
                if ii_on_act(c):
                    nc.scalar.activation(
                        out=act_scr[:], in_=it[:, j, :],
                        func=mybir.ActivationFunctionType.Square,
                        accum_out=ii_all[:, c : c + 1],
                    )
                else:
                    nc.vector.scalar_tensor_tensor(
                        out=dve_scr[:], in0=it[:, j, :], scalar=1.0, in1=it[:, j, :],
                        op0=mult, op1=mult,
                        accum_out=ii_all[:, c : c + 1],
                    )
                c += 1
            if c == split:
                tail_chunk(0, 0, split)

        # tail chunk 1: only the last slices' loss contributions; then
        # partition-reduce loss_p[128,2] via PE matmul with ones -> PSUM[2,1].
        tail_chunk(1, split, slices)
        psc = ppool.tile([2, 1], F32)
        nc.tensor.matmul(psc[:], lhsT=loss_p[:], rhs=ones[:],
                         start=True, stop=True)
        loss_sb = stats.tile([2, 1], F32)
        nc.vector.tensor_copy(loss_sb[:], psc[:])
        nc.sync.dma_start(out_d, loss_sb[:])

    nc.compile()
    return nc


_compiled = None


def _get_nc():
    global _compiled
    if _compiled is None:
        _compiled = _build()
    return _compiled


def _run(s, im, nc=None, **kw):
    """Shard, run on 8 cores, return BassKernelResults."""
    s = np.ascontiguousarray(np.asarray(s, dtype=np.float32))
    im = np.ascontiguousarray(np.asarray(im, dtype=np.float32))
    assert s.shape == (N, D) and im.shape == (N, D)
    if nc is None:
        nc = _get_nc()
    in_maps = [
        {"s": s[c * ROWS : (c + 1) * ROWS], "im": im[c * ROWS : (c + 1) * ROWS]}
        for c in range(N_CORES)
    ]
    bkr = run_bass_kernel_spmd(nc, in_maps, core_ids=list(range(N_CORES)), **kw)
    return bkr


def kernel(s, im, temp=None, **_):
    bkr = _run(s, im)
    total = np.float64(0.0)
    for r in bkr.results:
        total += r["out"].astype(np.float64).sum()
    return np.float32(total)


# revision 12
# speedup vs baseline: 1.4497x; 1.4497x over previous
"""Cosine-similarity loss kernel for Trainium2 (8 NeuronCores, SPMD).

loss = -sum_n dot(s_n, im_n) / (||s_n|| * ||im_n||)   for s, im in R^{65536 x 512}

Strategy (memory-bound, ~358 GB/s HBM per core):
  - Shard the 65536 rows across 8 cores (8192 rows each, 32 MB/core streamed).
  - Per 128-row slice (64 slices/core), three fused one-pass reductions:
      dot = sum_d s*im  -> VectorE scalar_tensor_tensor (s*1)*im, accum_out
      ss  = sum_d s*s   -> ScalarE activation(Square, accum_out)
      ii  = sum_d im*im -> split DVE/ACT to balance effective per-op cost
                           (DVE ~776ns incl accum-read, ACT ~1182ns)
  - Tail (all in the ACT boot/default table -- Square+Sqrt share set 3, so
    no ACT_TABLE_LOAD switches):
      rsq = 1/sqrt(ss*ii) via ACT Sqrt + DVE reciprocal_approx_fast
      loss_p[128,1] = -sum_c dot*rsq   (DVE STT accum)
      scalar = ones^T @ loss_p via PE matmul -> PSUM[1,1]  (partition reduce)
      DMA 4 bytes out (single descriptor -> single completion receipt,
      vs [128,1] = 16 engines' straggling HBM-write receipts ~9us).
  - Host sums the 8 per-core scalars.
"""

import numpy as np
from contextlib import ExitStack

import concourse.bacc as bacc
import concourse.bass as bass
import concourse.mybir as mybir
import concourse.tile as tile
from concourse.bass_utils import run_bass_kernel_spmd

N, D = 65536, 512
N_CORES = 8
ROWS = N // N_CORES          # 8192 rows per core
P = 128                      # SBUF partitions
F32 = mybir.dt.float32


def _build(
    rows=ROWS,
    # slices per DMA tile (1 slice = 128 rows = 256KB/tensor).  Small first
    # tiles start compute early; small last tiles shrink the post-DMA tail.
    seg_schedule=(1, 1, 2) + (4,) * 13 + (2, 2, 1, 1, 1, 1),
    bufs=10,
    # which slices' ii goes to ACT: 5/16 spread evenly (DVE ~733ns/op vs ACT
    # ~940ns/op incl accum-read -> balance at DVE 108 ops / ACT 84 ops), and
    # never on the final slice so the last-arriving tile finishes fastest.
    # 22/64 on ACT: DVE waits on the later-arriving im tile each pair, so it
    # idles ~4us more than ACT mid-stream; biasing 2 extra ii ops to ACT
    # equalizes the post-stream backlog (~5us each).
    ii_on_act=lambda c: c % 16 in (1, 4, 7, 10, 13) or c in (30, 62),
    im_dma="sync",           # engine issuing im loads: gpsimd | sync | scalar
                             # (scalar ring costs ~650ns/trigger on ACT: -47us;
                             # gpsimd/SWDGE starves vs DVE 2-port: both worse)
    mapping="pj",            # row->partition: jp = row j*128+p; pj = p*seg+j
                             # (pj gives contiguous per-partition DMA segments)
):
    slices = rows // P
    assert sum(seg_schedule) == slices

    nc = bacc.Bacc(
        "TRN2", target_bir_lowering=False, debug=False, num_devices=N_CORES
    )
    s_d = nc.dram_tensor("s", [rows, D], F32, kind="ExternalInput").ap()
    im_d = nc.dram_tensor("im", [rows, D], F32, kind="ExternalInput").ap()
    out_d = nc.dram_tensor("out", [2, 1], F32, kind="ExternalOutput").ap()

    mult = mybir.AluOpType.mult

    with tile.TileContext(nc) as tc, ExitStack() as ctx:
        spool = ctx.enter_context(tc.tile_pool(name="spool", bufs=bufs))
        ipool = ctx.enter_context(tc.tile_pool(name="ipool", bufs=bufs))
        stats = ctx.enter_context(tc.tile_pool(name="stats", bufs=1))
        ppool = ctx.enter_context(
            tc.tile_pool(name="psum", bufs=1, space=bass.MemorySpace.PSUM)
        )

        dot_all = stats.tile([P, slices], F32)
        ss_all = stats.tile([P, slices], F32)
        ii_all = stats.tile([P, slices], F32)
        ones = stats.tile([P, 1], F32)
        nc.vector.memset(ones[:], 1.0)
        # First ACT instruction is a Sqrt (sqrt(1)=1, harmless): the
        # act-table pass then loads the sqrt_and_others set (which also
        # holds Square) once at entry, instead of a second 1.28us
        # ACT_TABLE_LOAD right before the tail's real Sqrt.
        nc.scalar.activation(ones[:], ones[:], mybir.ActivationFunctionType.Sqrt)
        dve_scr = stats.tile([P, D], F32)
        act_scr = stats.tile([P, D], F32)

        # tail math split in two chunks: chunk 0 (slices 0..split-1) is
        # emitted mid-stream right after its accums, fully hidden under the
        # DMA stream; only chunk 1 (the last slices) runs after the stream.
        split = 48
        prod = stats.tile([P, slices], F32)
        sqp = stats.tile([P, slices], F32)
        rsq = stats.tile([P, slices], F32)
        fin_scr = stats.tile([P, slices], F32)
        loss_p = stats.tile([P, 2], F32)

        def tail_chunk(idx, lo, hi):
            w = slice(lo, hi)
            nc.vector.tensor_tensor(
                out=prod[:, w], in0=ss_all[:, w], in1=ii_all[:, w], op=mult
            )
            nc.scalar.activation(
                sqp[:, w], prod[:, w], mybir.ActivationFunctionType.Sqrt
            )
            nc.vector.reciprocal_approx_fast(rsq[:, w], sqp[:, w])
            nc.vector.scalar_tensor_tensor(
                out=fin_scr[:, w], in0=dot_all[:, w], scalar=-1.0, in1=rsq[:, w],
                op0=mult, op1=mult,
                accum_out=loss_p[:, idx : idx + 1],
            )

        c = 0
        r0 = 0
        pat = "(j p) d -> p j d" if mapping == "jp" else "(p j) d -> p j d"
        for seg in seg_schedule:
            nrows = seg * P
            s_seg = s_d[r0 : r0 + nrows, :].rearrange(pat, p=P, j=seg)
            im_seg = im_d[r0 : r0 + nrows, :].rearrange(pat, p=P, j=seg)
            r0 += nrows
            st = spool.tile([P, seg, D], F32, name="st", tag="st")
            nc.sync.dma_start(st[:], s_seg)
            it = ipool.tile([P, seg, D], F32, name="it", tag="it")
            getattr(nc, im_dma).dma_start(it[:], im_seg)
            for j in range(seg):
                nc.vector.scalar_tensor_tensor(
                    out=dve_scr[:], in0=st[:, j, :], scalar=1.0, in1=it[:, j, :],
                    op0=mult, op1=mult,
                    accum_out=dot_all[:, c : c + 1],
                )
                nc.scalar.activation(
                    out=act_scr[:], in_=st[:, j, :],
                    func=mybir.ActivationFunctionType.Square,
                    accum_out=ss_all[:, c : c + 1],
                )
                if ii_on_act(c):
                    nc.scalar.activation(
                        out=act_scr[:], in_=it[:, j, :],
                        func=mybir.ActivationFunctionType.Square,
                        accum_out=ii_all[:, c : c + 1],
                    )
                else:
                    nc.vector.scalar_tensor_tensor(
                        out=dve_scr[:], in0=it[:, j, :], scalar=1.0, in1=it[:, j, :],
                        op0=mult, op1=mult,
                        accum_out=ii_all[:, c : c + 1],
                    )
                c += 1
            if c == split:
                tail_chunk(0, 0, split)

        # tail chunk 1: only the last slices' loss contributions; then
        # partition-reduce loss_p[128,2] via PE matmul with ones -> PSUM[2,1].
        tail_chunk(1, split, slices)
        psc = ppool.tile([2, 1], F32)
        nc.tensor.matmul(psc[:], lhsT=loss_p[:], rhs=ones[:],
                         start=True, stop=True)
        loss_sb = stats.tile([2, 1], F32)
        nc.vector.tensor_copy(loss_sb[:], psc[:])
        nc.sync.dma_start(out_d, loss_sb[:])

    nc.compile()
    return nc


_compiled = None


def _get_nc():
    global _compiled
    if _compiled is None:
        _compiled = _build()
    return _compiled


def _run(s, im, nc=None, **kw):
    """Shard, run on 8 cores, return BassKernelResults."""
    s = np.ascontiguousarray(np.asarray(s, dtype=np.float32))
    im = np.ascontiguousarray(np.asarray(im, dtype=np.float32))
    assert s.shape == (N, D) and im.shape == (N, D)
    if nc is None:
        nc = _get_nc()
    in_maps = [
        {"s": s[c * ROWS : (c + 1) * ROWS], "im": im[c * ROWS : (c + 1) * ROWS]}
        for c in range(N_CORES)
    ]
    bkr = run_bass_kernel_spmd(nc, in_maps, core_ids=list(range(N_CORES)), **kw)
    return bkr


def kernel(s, im, temp=None, **_):
    bkr = _run(s, im)
    total = np.float64(0.0)
    for r in bkr.results:
        total += r["out"].astype(np.float64).sum()
    return np.float32(total)
